# revision 17
# baseline (speedup 1.0000x reference)
"""Trainium2 Bass kernel for CCHead (criss-cross attention head).

Self-contained: kernel(**inputs) takes the full unsharded inputs
(x[8, 2048, 64, 64] + weights), shards batch across 8 NeuronCores
(1 image per core, all params replicated), and returns the full
output [8, 104, 64, 64] float32.

v2: bf16 matmul operands throughout (FWL weight loads, no f32r
small-N penalty, half DMA), v-conv packed to M=128, activations kept
SBUF-resident between stages (ping-pong src buffers, no DRAM
roundtrips for CCA/convb outputs).
"""
import contextlib
import numpy as np
import ml_dtypes

import concourse.bass as bass
import concourse.tile as tile
from concourse import bacc, mybir

f32 = mybir.dt.float32
f32r = mybir.dt.float32r
bf16 = mybir.dt.bfloat16
AF = mybir.ActivationFunctionType
BF_NP = ml_dtypes.bfloat16

X_DEV_SHAPE = (16, 128, 64, 64)

S = 65
NR = 67
FLAT = NR * S + 2          # 4357
IMG0 = 1 + S               # flat offset of image row 0, col 0 = 66
STRIPS = [(r, 7) for r in range(0, 63, 7)] + [(63, 1)]
GROUPS = [STRIPS[0:2], STRIPS[2:4], STRIPS[4:6], STRIPS[6:8], STRIPS[8:10]]
GROUP_R0 = [0, 14, 28, 42, 56]
XS_FLAT = 16 * S + 3       # 1 lead pad + 16 rows * 65 + 2 slack
QK_TILES = [(i * 512, 512) for i in range(8)] + [(4096, 64)]


def host_prep(inputs):
    f = np.float32

    def fold(w, g, b, m, v):
        s = (g / np.sqrt(v + 1e-5)).astype(f)
        return (w * s[:, None, None, None]).astype(f), (b - m * s).astype(f)

    def wt_dev(w):  # [co, ci, 3, 3] -> [nci, 128, 9, co] bf16
        co, ci = w.shape[:2]
        return np.ascontiguousarray(
            w.reshape(co, ci, 9).transpose(1, 2, 0).reshape(ci // 128, 128, 9, co)
            .astype(BF_NP))

    def t1x1(w):  # [co, ci, 1, 1] -> [nci, 128, co] bf16
        co, ci = w.shape[:2]
        return np.ascontiguousarray(
            w.reshape(co, ci).T.reshape(ci // 128, 128, co).astype(BF_NP))

    G = np.array([[1, 0, 0], [.5, .5, .5], [.5, -.5, .5], [0, 0, 1]], f)

    def wino_u(w):  # [co, ci, 3, 3] -> [16, ci//128, 128, co] bf16
        co, ci = w.shape[:2]
        U = np.einsum('ar,bs,ocrs->abco', G, G, w.astype(f))      # [4,4,ci,co]
        return np.ascontiguousarray(
            U.reshape(16, ci // 128, 128, co).astype(BF_NP))

    wa, ba = fold(inputs['conva_w'], inputs['conva_g'], inputs['conva_b'],
                  inputs['conva_m'], inputs['conva_v'])
    wb, bb = fold(inputs['convb_w'], inputs['convb_g'], inputs['convb_b'],
                  inputs['convb_m'], inputs['convb_v'])
    wt, bt = fold(inputs['bott_w'], inputs['bott_g'], inputs['bott_b'],
                  inputs['bott_m'], inputs['bott_v'])
    gamma = float(np.asarray(inputs['cc_gamma']).reshape(-1)[0])
    mask = np.zeros((64, 64), f)
    np.fill_diagonal(mask, -1e30)
    dev = {
        'ua': wino_u(wa), 'ba': ba.reshape(4, 128, 1),
        'ub': wino_u(wb), 'bb': bb.reshape(4, 128, 1),
        'ut': wino_u(wt), 'bt': bt.reshape(4, 128, 1),
        'wc': t1x1(inputs['cls_w']), 'bc': inputs['cls_b'].astype(f).reshape(104, 1),
        'wq': t1x1(inputs['q_w']), 'bq': inputs['q_b'].astype(f).reshape(64, 1),
        'wk': t1x1(inputs['k_w']), 'bk': inputs['k_b'].astype(f).reshape(64, 1),
        'wv': t1x1(inputs['v_w']),
        'gvb': (gamma * inputs['v_b']).astype(f).reshape(4, 128, 1),
        'mask': mask,
        'zeros': np.zeros((128, 1056), BF_NP),
        'ident': np.eye(64, dtype=BF_NP),
    }
    return dev, gamma


INPUT_SPECS = [
    ('ua', [16, 16, 128, 512], bf16), ('ba', [4, 128, 1], f32),
    ('ub', [16, 4, 128, 512], bf16), ('bb', [4, 128, 1], f32),
    ('ut', [16, 20, 128, 512], bf16), ('bt', [4, 128, 1], f32),
    ('wc', [4, 128, 104], bf16), ('bc', [104, 1], f32),
    ('wq', [4, 128, 64], bf16), ('bq', [64, 1], f32),
    ('wk', [4, 128, 64], bf16), ('bk', [64, 1], f32),
    ('wv', [4, 128, 512], bf16),
    ('gvb', [4, 128, 1], f32),
    ('mask', [64, 64], f32),
    ('zeros', [128, 1056], bf16),
    ('ident', [64, 64], bf16),
]


def build(gamma, n_reps=1, debug=False):
    nc = bacc.Bacc("TRN2", num_devices=8)
    t = {'x': nc.dram_tensor("x", [16, 128, 64, 64], bf16, kind="ExternalInput")}
    for nm, shape, dt in INPUT_SPECS:
        t[nm] = nc.dram_tensor(nm, shape, dt, kind="ExternalInput")
    y = nc.dram_tensor("y", [104, 64, 64], f32, kind="ExternalOutput")
    t['o_t'] = nc.dram_tensor("o_t", [4, 128, 64, 64], bf16,
                              kind="ExternalOutput" if debug else "Internal")
    if debug:
        for nm in ['o_a', 'o_c1', 'o_c2', 'o_b']:
            t[nm] = nc.dram_tensor(nm, [4, 128, 64, 64], bf16,
                                   kind="ExternalOutput")
    with tile.TileContext(nc) as tc:
        _build_body(tc, t, y, gamma, n_reps, debug)
    nc.compile()
    return nc


def _rows(flat_tile):
    """[128, FLAT] -> padded row view [128, 67, 65] (skips lead pad elem)."""
    return flat_tile[:, 1:1 + NR * S].rearrange("p (r c) -> p r c", c=S)


def _build_body(tc, t, y, gamma, n_reps, debug):
    nc = tc.nc
    with contextlib.ExitStack() as est:
        cp = est.enter_context(tc.tile_pool(name="const", bufs=1))
        zeros = cp.tile([128, 1056], bf16)
        nc.sync.dma_start(zeros[:], t['zeros'][:])
        ident = cp.tile([64, 64], bf16)
        nc.sync.dma_start(ident[:], t['ident'][:])
        mask = cp.tile([64, 64], f32)
        nc.sync.dma_start(mask[:], t['mask'][:])

        def load_blocks(nm, n, shape, dt=f32):
            out = []
            for i in range(n):
                tl = cp.tile(shape, dt, tag=f"{nm}{i}", name=f"{nm}{i}")
                nc.sync.dma_start(tl[:], t[nm][i])
                out.append(tl)
            return out

        C = dict(nc=nc, tc=tc, t=t, y=y, gamma=gamma, zeros=zeros, ident=ident,
                 mask=mask,
                 bias_a=load_blocks('ba', 4, [128, 1]),
                 bias_b=load_blocks('bb', 4, [128, 1]),
                 bias_t=load_blocks('bt', 4, [128, 1]),
                 gvb=load_blocks('gvb', 4, [128, 1]),
                 wq=load_blocks('wq', 4, [128, 64], bf16),
                 wk=load_blocks('wk', 4, [128, 64], bf16),
                 wv=load_blocks('wv', 4, [128, 512], bf16),
                 wc=load_blocks('wc', 4, [128, 104], bf16),
                 debug=debug)
        for nm, p in [('bq', 64), ('bk', 64), ('bc', 104)]:
            C[nm] = cp.tile([p, 1], f32, tag=nm, name=nm)
            nc.sync.dma_start(C[nm][:], t[nm][:])

        ap = est.enter_context(tc.tile_pool(name="actp", bufs=1))
        srcA = [ap.tile([128, FLAT], bf16, tag=f"sA{i}", name=f"sA{i}") for i in range(4)]
        srcB = [ap.tile([128, FLAT], bf16, tag=f"sB{i}", name=f"sB{i}") for i in range(4)]
        for blk in srcA + srcB:
            _zero_act_borders(nc, blk, zeros)
        C['srcA'] = srcA
        C['srcB'] = srcB

        for _ in range(n_reps):
            _network(C)


def _zero_act_borders(nc, blk, zeros):
    rv = _rows(blk)
    nc.sync.dma_start(blk[:, 0:1], zeros[:, 0:1])               # lead pad
    nc.sync.dma_start(blk[:, FLAT - 1:FLAT], zeros[:, 0:1])     # slack
    nc.sync.dma_start(rv[:, :, 64:65], zeros[:, 0:NR].unsqueeze(2))
    nc.sync.dma_start(rv[:, 0:1, 0:64], zeros[:, 0:64].unsqueeze(1))
    nc.sync.dma_start(rv[:, 65:67, 0:64],
                      zeros[:, 0:128].rearrange("p (r c) -> p r c", c=64))


def _dump_src(C, src, dram):
    nc = C['nc']
    for cb in range(4):
        nc.sync.dma_start(dram[cb], _rows(src[cb])[:, 1:65, 0:64])


def _network(C):
    nc, tc, t = C['nc'], C['tc'], C['t']
    srcA, srcB = C['srcA'], C['srcB']
    # conva: x -> srcA (SBUF resident), Winograd
    with contextlib.ExitStack() as es:
        xwp = es.enter_context(tc.tile_pool(name="xwp", bufs=3))
        xg = _x_wino_getter(C, xwp)

        def dst_a(co, h, acc_ap):
            dst = _rows(srcA[co])[:, 1 + 32 * h:1 + 32 * h + 32, 0:64]
            nc.scalar.activation(dst, acc_ap, AF.Relu, bias=C['bias_a'][co],
                                 scale=1.0)

        _conv_wino(C, 16, t['ua'], C['bias_a'], xg, dst_a)
    if C['debug']:
        _dump_src(C, srcA, t['o_a'])
    # CCA 1: srcA -> srcB
    _cca(C, srcA, srcB)
    if C['debug']:
        _dump_src(C, srcB, t['o_c1'])
    # CCA 2: srcB -> srcA
    _cca(C, srcB, srcA)
    if C['debug']:
        _dump_src(C, srcA, t['o_c2'])
    # convb: srcA -> srcB, Winograd
    sga = _src_wino_getter(srcA)

    def dst_b(co, h, acc_ap):
        dst = _rows(srcB[co])[:, 1 + 32 * h:1 + 32 * h + 32, 0:64]
        nc.scalar.activation(dst, acc_ap, AF.Relu, bias=C['bias_b'][co],
                             scale=1.0)

    _conv_wino(C, 4, t['ub'], C['bias_b'],
               lambda h, a, cb, kr: sga(h, a, cb), dst_b)
    if C['debug']:
        _dump_src(C, srcB, t['o_b'])
    # bott: x strips (16cb) + srcB (4cb) -> SBUF o_t; cls reads it directly
    with contextlib.ExitStack() as es:
        otp = es.enter_context(tc.tile_pool(name="otp", bufs=1))
        ot_sb = [otp.tile([128, 64, 64], bf16, tag=f"ot{i}", name=f"ot{i}")
                 for i in range(4)]
        with contextlib.ExitStack() as esb:
            xwp = esb.enter_context(tc.tile_pool(name="xwpt", bufs=3))
            xg = _x_wino_getter(C, xwp)
            sgb = _src_wino_getter(srcB)

            def src_get(h, a, cb, kr):
                return xg(h, a, cb, kr) if cb < 16 else sgb(h, a, cb - 16)

            def dst_t(co, h, acc_ap):
                nc.scalar.activation(ot_sb[co][:, 32 * h:32 * h + 32, :],
                                     acc_ap, AF.Relu, bias=C['bias_t'][co],
                                     scale=1.0)

            _conv_wino(C, 20, t['ut'], C['bias_t'], src_get, dst_t)
        if C['debug']:
            for cb in range(4):
                nc.sync.dma_start(t['o_t'][cb], ot_sb[cb][:])
        # cls: read o_t from SBUF -> y
        with contextlib.ExitStack() as esc:
            cop = esc.enter_context(tc.tile_pool(name="cop", bufs=1))
            cpp = esc.enter_context(tc.tile_pool(name="cpp", bufs=2, space="PSUM"))
            out_sb = cop.tile([104, 64, 64], f32)
            oflat = out_sb[:].rearrange("p r c -> p (r c)")
            for off, n in [(i * 512, 512) for i in range(8)]:
                ps = cpp.tile([104, 512], f32, tag="clsps")
                for cb in range(4):
                    rhs = ot_sb[cb][:].rearrange("p r c -> p (r c)")[:, off:off + n]
                    nc.tensor.matmul(ps[:, 0:n], C['wc'][cb][:], rhs,
                                     start=(cb == 0), stop=(cb == 3))
                nc.scalar.activation(oflat[:, off:off + n], ps[:, 0:n], AF.Identity,
                                     bias=C['bc'][:], scale=1.0)
            nc.sync.dma_start(C['y'][:], out_sb[:])


def _x_strip_getter(C, xsp):
    nc, zeros, t = C['nc'], C['zeros'], C['t']
    cache = {}

    def get(g, cb):
        key = (g, cb)
        if key in cache:
            return cache[key]
        xs = xsp.tile([128, XS_FLAT], bf16, tag="xs")
        rv = xs[:, 1:1 + 16 * S].rearrange("p (r c) -> p r c", c=S)
        r0g = GROUP_R0[g]
        lo = max(0, r0g - 1)
        hi = min(64, r0g + 15)
        l0, l1 = lo - (r0g - 1), lo - (r0g - 1) + hi - lo
        nc.sync.dma_start(xs[:, 0:1], zeros[:, 0:1])
        nc.sync.dma_start(xs[:, XS_FLAT - 2:XS_FLAT], zeros[:, 0:2])
        nc.sync.dma_start(rv[:, :, 64:65], zeros[:, 0:16].unsqueeze(2))
        if l0 > 0:
            nc.sync.dma_start(rv[:, 0:l0, 0:64],
                              zeros[:, 0:l0 * 64].rearrange("p (r c) -> p r c", c=64))
        if l1 < 16:
            nc.sync.dma_start(rv[:, l1:16, 0:64],
                              zeros[:, 0:(16 - l1) * 64].rearrange("p (r c) -> p r c", c=64))
        nc.sync.dma_start(rv[:, l0:l1, 0:64], t['x'][cb][:, lo:hi, :])
        res = (xs, lambda r0, _g=r0g: r0 - _g + 1)
        cache[key] = res
        return res

    return get


def _act_src_getter(C, src):
    def get(g, cb):
        return (src[cb], lambda r0: r0 + 1)
    return get



# ---- Winograd F(2x2, 3x3) ----
# output tile (2x2) grid is 32x32; halves split tile rows 0:16 / 16:32.
# B^T row combos (operands indexed by a = row offset -1,0,1,2):
#   t0 = d(-1) - d(1);  t1 = d(0) + d(1);  t2 = d(1) - d(0);  t3 = d(0) - d(2)
_T_OPS = {0: ('sub', -1, 1), 1: ('add', 0, 1), 2: ('sub', 1, 0), 3: ('sub', 0, 2)}
# A^T columns: output row offset a contributions per kr: (a, sign)
_AT_TERMS = {0: [(0, 1)], 1: [(0, 1), (1, 1)], 2: [(0, 1), (1, -1)], 3: [(1, -1)]}
_FIRST_KR = {0: 0, 1: 1}  # first kr contributing to output offset a


def _src_wino_getter(src):
    """Operand provider for SBUF-resident padded src blocks.

    get(h, a, cb) -> AP [128, 16, 66] covering rows 2*(16h+i)+a, cols -1..64.
    """
    def get(h, a, cb):
        off0 = 65 * (32 * h + a + 1)
        return (src[cb][:, off0:off0 + 2080]
                .rearrange("p (i c) -> p i c", c=130)[:, :, 0:66])
    return get


def _x_wino_getter(C, xwp):
    """Operand provider staging x rows from DRAM per (h, kr, cb).

    Stages minimal parity rows into zero-bordered [128, nr, 66] tiles.
    """
    nc, zeros, t = C['nc'], C['zeros'], C['t']
    state = {}

    def stage(h, kr, cb):
        key = (h, kr, cb)
        if key in state:
            return state[key]
        if kr == 0:          # odd rows 32h-1 .. 32h+31 (17)
            nr, kind = 17, 'odd'
        elif kr in (1, 2):   # rows 32h .. 32h+31 (32)
            nr, kind = 32, 'all'
        else:                # even rows 32h .. 32h+32 (17)
            nr, kind = 17, 'even'
        xw = xwp.tile([128, nr, 66], bf16, tag=f"xw{nr}")
        nc.sync.dma_start(xw[:, :, 0:1], zeros[:, 0:nr].unsqueeze(2))
        nc.sync.dma_start(xw[:, :, 65:66], zeros[:, 0:nr].unsqueeze(2))
        xd = t['x'][cb]
        if kind == 'all':
            nc.sync.dma_start(xw[:, :, 1:65], xd[:, 32 * h:32 * h + 32, :])
        else:
            par = 1 if kind == 'odd' else 0
            xv = xd.rearrange("p (r2 two) c -> p r2 two c", two=2)[:, :, par, :]
            k0 = 16 * h - 1 if kind == 'odd' else 16 * h
            k1 = k0 + 17
            lo, hi = max(k0, 0), min(k1, 32)
            i0, i1 = lo - k0, lo - k0 + hi - lo
            if i0 > 0:
                nc.sync.dma_start(xw[:, 0:i0, 1:65],
                                  zeros[:, 0:i0 * 64].rearrange("p (r c) -> p r c", c=64))
            if i1 < 17:
                nc.sync.dma_start(xw[:, i1:17, 1:65],
                                  zeros[:, 0:(17 - i1) * 64].rearrange("p (r c) -> p r c", c=64))
            nc.sync.dma_start(xw[:, i0:i1, 1:65], xv[:, lo:hi, :])
        state[key] = xw
        return xw

    def get(h, a, cb, kr):
        xw = stage(h, kr, cb)
        if kr == 0:          # tile rows: a=-1 -> 0:16, a=1 -> 1:17
            s0 = 0 if a == -1 else 1
            return xw[:, s0:s0 + 16, :]
        if kr in (1, 2):     # a in (0,1): parity view
            return xw[:].rearrange("p (i2 two) c -> p i2 two c", two=2)[:, :, a, :]
        s0 = 0 if a == 0 else 1
        return xw[:, s0:s0 + 16, :]
    return get


def _conv_wino(C, n_cb, u_dram, bias_sb, src_get, dst_act):
    """F(2x2,3x3) conv: n_cb input channel blocks -> 4 co blocks.

    src_get(h, a, cb, kr) -> [128, 16, 66] operand AP for d(2*(16h+i)+a).
    dst_act(co, h, acc_ap) called at end of each half with the f32 accum.
    """
    nc, tc = C['nc'], C['tc']
    with contextlib.ExitStack() as es:
        tp = es.enter_context(tc.tile_pool(name="wtp", bufs=1))
        dap = es.enter_context(tc.tile_pool(name="wdap", bufs=1))
        vp = es.enter_context(tc.tile_pool(name="wvp", bufs=4))
        up = es.enter_context(tc.tile_pool(name="wup", bufs=2))
        pp = es.enter_context(tc.tile_pool(name="wpp", bufs=2, space="PSUM"))
        for h in range(2):
            acc = {co: dap.tile([128, 32, 64], f32, tag=f"da{co}", name=f"da{co}")
                   for co in range(4)}
            for kr in range(4):
                op, a0, a1 = _T_OPS[kr]
                ts = {}
                for cb in range(n_cb):
                    tt = tp.tile([128, 16, 66], bf16, tag=f"t{cb}", name=f"wt{cb}")
                    A = src_get(h, a0, cb, kr)
                    B = src_get(h, a1, cb, kr)
                    if op == 'add':
                        nc.vector.tensor_add(tt[:], A, B)
                    else:
                        nc.vector.tensor_sub(tt[:], A, B)
                    ts[cb] = tt
                for kc in range(4):
                    p = kr * 4 + kc
                    pt = {co: pp.tile([128, 512], f32, tag=f"wg{co}", name=f"wg{co}")
                          for co in range(4)}
                    for cb in range(n_cb):
                        tv = ts[cb][:].rearrange("p i (c2 two) -> p i c2 two", two=2)
                        V = vp.tile([128, 16, 32], bf16, tag="V")
                        if kc == 0:
                            nc.vector.tensor_sub(V[:], tv[:, :, 0:32, 0], tv[:, :, 1:33, 0])
                        elif kc == 1:
                            nc.vector.tensor_add(V[:], tv[:, :, 0:32, 1], tv[:, :, 1:33, 0])
                        elif kc == 2:
                            nc.vector.tensor_sub(V[:], tv[:, :, 1:33, 0], tv[:, :, 0:32, 1])
                        else:
                            nc.vector.tensor_sub(V[:], tv[:, :, 0:32, 1], tv[:, :, 1:33, 1])
                        uw = up.tile([128, 512], bf16, tag="uw")
                        nc.sync.dma_start(uw[:], u_dram[p, cb])
                        vflat = V[:].rearrange("p i c -> p (i c)")
                        for co in range(4):
                            nc.tensor.matmul(pt[co][:],
                                             uw[:, co * 128:(co + 1) * 128],
                                             vflat,
                                             start=(cb == 0), stop=(cb == n_cb - 1))
                    for co in range(4):
                        mv = pt[co][:].rearrange("p (i j) -> p i j", j=32)
                        dv = acc[co][:].rearrange(
                            "p (i two) (j twoc) -> p i two j twoc", two=2, twoc=2)
                        for (a, sa) in _AT_TERMS[kr]:
                            for (b, sb) in _AT_TERMS[kc]:
                                d_sl = dv[:, :, a, :, b]
                                if kr == _FIRST_KR[a] and kc == _FIRST_KR[b]:
                                    nc.vector.tensor_copy(d_sl, mv)
                                elif sa * sb > 0:
                                    nc.vector.tensor_add(d_sl, d_sl, mv)
                                else:
                                    nc.vector.tensor_sub(d_sl, d_sl, mv)
            for co in range(4):
                dst_act(co, h, acc[co][:])


def _conv3x3(C, wp, cps, src_getter, n_cb, w_dram, bias_sb,
             dst_sbuf=None, dst_dram=None, dst_plain=None, stage_pool=None):
    nc = C['nc']
    for g, strips in enumerate(GROUPS):
        psums = {}
        for si, (r0, nr) in enumerate(strips):
            for co in range(4):
                psums[(si, co)] = cps.tile([128, nr * S + 1], f32, tag=f"c{si}{co}", name=f"c{si}{co}")
        for cb in range(n_cb):
            wtl = wp.tile([128, 9, 512], bf16, tag="w")
            nc.sync.dma_start(wtl[:], w_dram[cb])
            sflat, base_row = src_getter(g, cb)
            for tap in range(9):
                dy, dx = tap // 3 - 1, tap % 3 - 1
                for co in range(4):
                    for si, (r0, nr) in enumerate(strips):
                        n = nr * S + 1
                        off = 1 + (base_row(r0) + dy) * S + dx
                        nc.tensor.matmul(
                            psums[(si, co)][:],
                            wtl[:, tap, co * 128:(co + 1) * 128],
                            sflat[:, off:off + n],
                            start=(cb == 0 and tap == 0),
                            stop=(cb == n_cb - 1 and tap == 8))
        for si, (r0, nr) in enumerate(strips):
            for co in range(4):
                ps = psums[(si, co)]
                pv = ps[:, 0:nr * S].rearrange("p (r c) -> p r c", c=S)[:, :, 0:64]
                if dst_sbuf is not None:
                    dst = _rows(dst_sbuf[co])[:, 1 + r0:1 + r0 + nr, 0:64]
                    nc.scalar.activation(dst, pv, AF.Relu, bias=bias_sb[co], scale=1.0)
                elif dst_plain is not None:
                    nc.scalar.activation(dst_plain[co][:, r0:r0 + nr, :], pv,
                                         AF.Relu, bias=bias_sb[co], scale=1.0)
                else:
                    stg = stage_pool.tile([128, 7, 64], bf16, tag="cstg")
                    nc.scalar.activation(stg[:, 0:nr, :], pv, AF.Relu,
                                         bias=bias_sb[co], scale=1.0)
                    nc.sync.dma_start(dst_dram[co][:, r0:r0 + nr, :], stg[:, 0:nr, :])


def _cca(C, src, dst):
    """One criss-cross attention: dst = gamma*(outh+outw+v_b) + src.

    src, dst: lists of 4 SBUF-resident [128, FLAT] bf16 blocks (padded
    layout). dst image region is fully overwritten; borders stay zero.
    """
    nc, tc = C['nc'], C['tc']
    gamma, ident, mask = C['gamma'], C['ident'], C['mask']
    with contextlib.ExitStack() as es:
        atp = es.enter_context(tc.tile_pool(name="atp", bufs=1))
        ATh = atp.tile([64, 64, 64], bf16, tag="ATh")
        ATw = atp.tile([64, 64, 64], bf16, tag="ATw")
        # ---- phase A: q/k convs + energies -> EA (f32) -> softmax -> AT (bf16)
        with contextlib.ExitStack() as esA:
            qkp = esA.enter_context(tc.tile_pool(name="qkp", bufs=1))
            eap = esA.enter_context(tc.tile_pool(name="eap", bufs=1))
            smp = esA.enter_context(tc.tile_pool(name="smp", bufs=3))
            stp = esA.enter_context(tc.tile_pool(name="stp", bufs=2))
            psA = esA.enter_context(tc.tile_pool(name="psA", bufs=2, space="PSUM"))
            psB = esA.enter_context(tc.tile_pool(name="psB", bufs=3, space="PSUM"))
            q_sb = qkp.tile([64, 64, 65], bf16, tag="q")
            k_sb = qkp.tile([64, 64, 65], bf16, tag="k")
            EA = eap.tile([64, 64, 128], f32, tag="EA")
            EAb = eap.tile([64, 64, 128], bf16, tag="EAb")
            for dst_sb, wgt, bias in [(q_sb, C['wq'], C['bq']),
                                      (k_sb, C['wk'], C['bk'])]:
                dflat = dst_sb[:].rearrange("p r c -> p (r c)")
                for off, n in QK_TILES:
                    ps = psA.tile([64, 512], f32, tag="qkps")
                    for cb in range(4):
                        rhs = src[cb][:, IMG0 + off:IMG0 + off + n]
                        nc.tensor.matmul(ps[:, 0:n], wgt[cb][:], rhs,
                                         start=(cb == 0), stop=(cb == 3))
                    nc.scalar.activation(dflat[:, off:off + n], ps[:, 0:n],
                                         AF.Identity, bias=bias[:], scale=1.0)
            for w in range(64):
                ps = psB.tile([64, 64], f32, tag="e64")
                nc.tensor.matmul(ps[:], q_sb[:, :, w], k_sb[:, :, w],
                                 start=True, stop=True)
                nc.vector.tensor_add(EA[:, w, 0:64], ps[:], mask[:])
            for h in range(64):
                ps = psB.tile([64, 64], f32, tag="e64")
                nc.tensor.matmul(ps[:], q_sb[:, h, 0:64], k_sb[:, h, 0:64],
                                 start=True, stop=True)
                st = stp.tile([64, 64], f32, tag="st")
                nc.vector.tensor_copy(st[:], ps[:])
                nc.sync.dma_start(EA[h:h + 1, :, 64:128], st[:])
            # ---- softmax (f32) -> EAb (bf16) -> transposes -> ATh/ATw (bf16)
            for w in range(64):
                rs = smp.tile([64, 1], f32, tag="rs")
                nc.scalar.activation(EA[:, w, :], EA[:, w, :], AF.Exp,
                                     accum_out=rs[:])
                ri = smp.tile([64, 1], f32, tag="ri")
                nc.vector.reciprocal(ri[:], rs[:])
                nc.scalar.activation(EAb[:, w, :], EA[:, w, :], AF.Copy,
                                     scale=ri[:])
                psh = psB.tile([64, 64], bf16, tag="e64b")
                nc.tensor.transpose(psh[:], EAb[:, w, 0:64], ident[:])
                nc.scalar.activation(ATh[:, w, :], psh[:], AF.Copy)
                psw = psB.tile([64, 64], bf16, tag="e64b")
                nc.tensor.transpose(psw[:], EAb[:, w, 64:128], ident[:])
                nc.scalar.activation(ATw[:, w, :], psw[:], AF.Copy)
        # ---- phase C: V (packed pairs, M=128) + aggregation -> dst
        with contextlib.ExitStack() as esC:
            vtp = esC.enter_context(tc.tile_pool(name="vtp", bufs=2))
            sgp = esC.enter_context(tc.tile_pool(name="sgp", bufs=3))
            psC = esC.enter_context(tc.tile_pool(name="psC", bufs=2, space="PSUM"))
            psD = esC.enter_context(tc.tile_pool(name="psD", bufs=4, space="PSUM"))
            # w-phase: dst = src + gamma*out_h
            for wc in range(16):
                ws = list(range(wc * 4, wc * 4 + 4))
                # VT for 4 w's as 2 packed pair-matmuls: [128=(w2,h), 512]
                VT = vtp.tile([64, 4, 512], bf16, tag="VT")
                for j in range(2):
                    w0 = ws[2 * j]
                    ps = psC.tile([128, 512], f32, tag="vps")
                    for cb in range(4):
                        stg = sgp.tile([128, 2, 64], bf16, tag="vstg")
                        nc.vector.tensor_copy(
                            stg[:], _rows(src[cb])[:, 1:65, w0:w0 + 2]
                            .rearrange("p r c -> p c r"))
                        nc.tensor.matmul(
                            ps[:], stg[:].rearrange("p c r -> p (c r)"),
                            C['wv'][cb][:],
                            start=(cb == 0), stop=(cb == 3))
                    nc.scalar.activation(VT[:, 2 * j, :], ps[0:64, :], AF.Copy)
                    nc.scalar.activation(VT[:, 2 * j + 1, :], ps[64:128, :],
                                         AF.Copy)
                for cb in range(4):
                    pso = psD.tile([128, 4, 64], f32, tag="ops")
                    for i, w in enumerate(ws):
                        nc.tensor.matmul(
                            pso[:, i, :],
                            VT[:, i, cb * 128:(cb + 1) * 128],
                            ATh[:, w, :], start=True, stop=True)
                    stg = sgp.tile([128, 4, 64], bf16, tag="stg")
                    nc.scalar.activation(stg[:], pso[:], AF.Copy, scale=gamma)
                    o_sl = _rows(dst[cb])[:, 1:65, wc * 4:wc * 4 + 4]
                    i_sl = _rows(src[cb])[:, 1:65, wc * 4:wc * 4 + 4]
                    nc.vector.tensor_add(o_sl, i_sl,
                                         stg[:].rearrange("p w h -> p h w"))
            # h-phase: dst += gamma*out_w + gamma*v_b
            for hc in range(16):
                hs = list(range(hc * 4, hc * 4 + 4))
                VT = vtp.tile([64, 4, 512], bf16, tag="VT")
                for j in range(2):
                    h0 = hs[2 * j]
                    ps = psC.tile([128, 512], f32, tag="vps")
                    for cb in range(4):
                        stg = sgp.tile([128, 2, 64], bf16, tag="vstg")
                        nc.vector.tensor_copy(
                            stg[:], _rows(src[cb])[:, h0 + 1:h0 + 3, 0:64])
                        nc.tensor.matmul(
                            ps[:], stg[:].rearrange("p c r -> p (c r)"),
                            C['wv'][cb][:],
                            start=(cb == 0), stop=(cb == 3))
                    nc.scalar.activation(VT[:, 2 * j, :], ps[0:64, :], AF.Copy)
                    nc.scalar.activation(VT[:, 2 * j + 1, :], ps[64:128, :],
                                         AF.Copy)
                for cb in range(4):
                    pso = psD.tile([128, 4, 64], f32, tag="ops")
                    for i, h in enumerate(hs):
                        nc.tensor.matmul(
                            pso[:, i, :],
                            VT[:, i, cb * 128:(cb + 1) * 128],
                            ATw[:, :, h], start=True, stop=True)
                    stg = sgp.tile([128, 4, 64], bf16, tag="stg")
                    nc.scalar.activation(stg[:], pso[:], AF.Identity,
                                         scale=gamma, bias=C['gvb'][cb][:])
                    o_sl = _rows(dst[cb])[:, 1 + hc * 4:1 + hc * 4 + 4, 0:64]
                    nc.vector.tensor_add(o_sl, o_sl, stg[:])


_BUILD_CACHE = {}


def _get_nc(gamma):
    key = round(float(gamma), 12)
    if key not in _BUILD_CACHE:
        _BUILD_CACHE[key] = build(gamma, n_reps=1)
    return _BUILD_CACHE[key]


def kernel(**inputs):
    from concourse.bass_utils import run_bass_kernel_spmd
    inputs_np = {k: np.asarray(v) for k, v in inputs.items()}
    dev, gamma = host_prep(inputs_np)
    nc = _get_nc(gamma)
    in_maps = []
    for core in range(8):
        m = dict(dev)
        m['x'] = np.ascontiguousarray(
            inputs_np['x'][core].astype(BF_NP).reshape(*X_DEV_SHAPE))
        in_maps.append(m)
    res = run_bass_kernel_spmd(nc, in_maps, core_ids=list(range(8)))
    out = np.stack([r['y'].reshape(104, 64, 64) for r in res.results])
    return out.astype(np.float32)
